# revision 1
# baseline (speedup 1.0000x reference)
"""Two-layer GCN (PyG GCNConv x2 + ReLU) on 8 Trainium2 NeuronCores.

Strategy (dst-sharded, SPMD single program):
  - Nodes padded to 102400, sharded 12800/core by destination.
  - Per layer: dense h = x_shard @ W on each core -> AllGather full h ->
    windowed dma_gather of h[src] per edge (4 windows of 25600 rows so the
    int16 gather indices fit) -> scatter-add via per-tile selection-matrix
    matmuls (S[e, dst_local] = norm_e, built on host) accumulated in PSUM
    -> SBUF accumulator -> ReLU epilogue.
  - Layer 1 runs "transposed" (psum[f, d] via lhsT=msg) so its output is
    directly the lhsT operand for layer 2's dense matmul; layer 2 runs
    normal (psum[d, f]) so the final output is row-major node x feature.
  - Symmetric norm a[src]*a[dst] is folded into S; biases: b1 is applied as
    the ACT per-partition bias in the transposed world; b2 via a broadcast
    add only when nonzero (it is zero in this problem's spec).
"""

import numpy as np

import concourse.bass as bass
import concourse.bacc as bacc
import concourse.mybir as mybir
import concourse.tile as tile
from concourse.bass_utils import run_bass_kernel_spmd

N = 100000
E = 640000
D = 128
NCORES = 8
NPAD = 102400
SHARD = NPAD // NCORES        # 12800
NBLK = SHARD // 128           # 100 dst blocks per core
WIN = 25600                   # gather window rows (int16-safe)
NW = NPAD // WIN              # 4 windows
CHUNK_T = 8                   # tiles (of 128 edges) per dma_gather call (1024 idx: ring limit)

_CACHE = {}


def _host_prep(x, edge_index, W1, b1, W2, b2):
    x = np.asarray(x, dtype=np.float32)
    ei = np.asarray(edge_index)
    W1 = np.asarray(W1, dtype=np.float32)
    W2 = np.asarray(W2, dtype=np.float32)
    b1 = np.asarray(b1, dtype=np.float32)
    b2 = np.asarray(b2, dtype=np.float32)
    n = x.shape[0]

    src = np.concatenate([ei[0], np.arange(n, dtype=np.int64)])
    dst = np.concatenate([ei[1], np.arange(n, dtype=np.int64)])
    deg = np.bincount(dst, minlength=NPAD).astype(np.float32)
    a = np.zeros(NPAD, np.float32)
    nz = deg > 0
    a[nz] = 1.0 / np.sqrt(deg[nz])

    # degree-balanced node->position permutation: deal nodes (sorted by degree
    # desc) round-robin over the 800 (core, block) pairs so every block has a
    # near-equal edge count; all device-side structures live in position space.
    order_by_deg = np.argsort(-deg, kind="stable")
    i = np.arange(NPAD, dtype=np.int64)
    cb = i % (NCORES * NBLK)
    position_of_rank = (cb % NCORES) * SHARD + (cb // NCORES) * 128 + i // (NCORES * NBLK)
    pos_of_node = np.empty(NPAD, np.int64)
    pos_of_node[order_by_deg] = position_of_rank
    node_at_pos = np.empty(NPAD, np.int64)
    node_at_pos[pos_of_node] = np.arange(NPAD, dtype=np.int64)

    ps = pos_of_node[src]
    pd = pos_of_node[dst]
    core = pd // SHARD
    # logical gather window per edge: 0 = the appended self-loop window
    # (served from the core-local dense output, so int16-indexable and free of
    # the clustering the node permutation would otherwise cause), 1..NW = the
    # four 25600-row slices of the AllGathered table.
    is_self = np.zeros(src.shape[0], bool)
    is_self[E if src.shape[0] == E + n else src.shape[0] - n:] = True
    NWG = NW + 1

    per_core = []
    counts_all = np.zeros((NCORES, NWG * NBLK), np.int64)
    for k in range(NCORES):
        m = core == k
        s_k = ps[m]
        d_k = pd[m]
        n_s = src[m]
        n_d = dst[m]
        self_k = is_self[m]
        w_k = np.where(self_k, 0, 1 + s_k // WIN)
        b_k = (d_k % SHARD) // 128
        key = w_k * NBLK + b_k
        order = np.lexsort((s_k, key))
        s_k, d_k, key = s_k[order], d_k[order], key[order]
        n_s, n_d, self_k = n_s[order], n_d[order], self_k[order]
        counts = np.bincount(key, minlength=NWG * NBLK)
        counts_all[k] = counts
        per_core.append((s_k, d_k, n_s, n_d, self_k, key, counts))

    # common tile schedule: T[w*NBLK+b] tiles of 128 edges, identical on all cores
    T = (np.max(counts_all, axis=0) + 127) // 128
    T[:NBLK] = np.maximum(T[:NBLK], 1)  # self window groups init the accumulator
    tile_base = np.zeros(NWG * NBLK + 1, np.int64)
    tile_base[1:] = np.cumsum(T)
    t_total = int(tile_base[-1])

    # gather-call schedule: chunks of <= CHUNK_T tiles, never crossing windows
    calls = []  # (window, tile_start, n_tiles); window 0 = local/self
    for w in range(NWG):
        w_start = int(tile_base[w * NBLK])
        w_end = int(tile_base[(w + 1) * NBLK])
        t = w_start
        while t < w_end:
            nt = min(CHUNK_T, w_end - t)
            calls.append((w, t, nt))
            t += nt

    # per-core padded flat arrays in tile order
    in_maps = []
    x_pad = np.zeros((NPAD, D), np.float32)
    x_pad[:n] = x
    x_perm = x_pad[node_at_pos]
    b2_nonzero = bool(np.any(b2 != 0.0))
    for k in range(NCORES):
        s_k, d_k, n_s, n_d, self_k, key, counts = per_core[k]
        ne = s_k.shape[0]
        grp_off = np.zeros(NWG * NBLK + 1, np.int64)
        grp_off[1:] = np.cumsum(counts)
        rank = np.arange(ne, dtype=np.int64) - grp_off[key]
        pos = tile_base[key] * 128 + rank

        gidx = np.zeros(t_total * 128, np.int16)
        norm = np.zeros(t_total * 128, np.float32)
        dloc = np.zeros(t_total * 128, np.int64)
        # self edges index the local bounce (position % SHARD); real edges
        # index their 25600-row window of the gathered table.
        rel = np.where(self_k, s_k % SHARD, s_k - (s_k // WIN) * WIN)
        gidx[pos] = rel.astype(np.int16)
        norm[pos] = a[n_s] * a[n_d]
        dloc[pos] = d_k % 128

        # S tiles: [128 e, t, 128 d] flattened to [128, t_total*128]
        S = np.zeros((t_total, 128, 128), np.float32)
        tt = np.arange(t_total * 128) // 128
        ee = np.arange(t_total * 128) % 128
        S[tt, ee, dloc] = norm
        S_t = np.ascontiguousarray(S.transpose(1, 0, 2).reshape(128, t_total * 128))

        # wrapped gather indices per call, replicated across the 8 Q7 groups
        idxw = np.zeros((128, t_total * 8), np.int16)
        for (w, t0, nt) in calls:
            blk = gidx[t0 * 128:(t0 + nt) * 128].reshape(nt * 8, 16).T
            idxw[:, t0 * 8:(t0 + nt) * 8] = np.tile(blk, (8, 1))

        xT = np.ascontiguousarray(x_perm[k * SHARD:(k + 1) * SHARD].T)
        in_maps.append({
            "xT": xT,
            "S": S_t,
            "idxw": idxw,
            "W1": W1,
            "W2": W2,
            "b1col": b1.reshape(128, 1).copy(),
            "b2bc": np.broadcast_to(b2, (128, 128)).copy(),
        })

    sched_sig = (tuple(int(v) for v in T), tuple(calls), b2_nonzero)
    return in_maps, sched_sig, tuple(int(v) for v in tile_base), t_total, b2_nonzero, pos_of_node


def _build_program(tile_base, t_total, calls, b2_nonzero):
    nc = bacc.Bacc("TRN2", target_bir_lowering=False, debug=False,
                   num_devices=NCORES, num_swdge_queues=4)
    f32 = mybir.dt.float32
    xT_d = nc.dram_tensor("xT", [D, SHARD], f32, kind="ExternalInput")
    S_d = nc.dram_tensor("S", [128, t_total * 128], f32, kind="ExternalInput")
    idx_d = nc.dram_tensor("idxw", [128, t_total * 8], mybir.dt.int16, kind="ExternalInput")
    W1_d = nc.dram_tensor("W1", [D, D], f32, kind="ExternalInput")
    W2_d = nc.dram_tensor("W2", [D, D], f32, kind="ExternalInput")
    b1_d = nc.dram_tensor("b1col", [128, 1], f32, kind="ExternalInput")
    b2_d = nc.dram_tensor("b2bc", [128, 128], f32, kind="ExternalInput")
    out_d = nc.dram_tensor("out", [SHARD, D], f32, kind="ExternalOutput")

    h1_bounce = nc.dram_tensor("h1_bounce", [SHARD, D], f32)
    h1_full = nc.dram_tensor("h1_full", [NPAD, D], f32, addr_space="Shared")
    h2_bounce = nc.dram_tensor("h2_bounce", [SHARD, D], f32)
    h2_full = nc.dram_tensor("h2_full", [NPAD, D], f32, addr_space="Shared")

    with tile.TileContext(nc) as tc:
        with (
            tc.tile_pool(name="const", bufs=1) as p_const,
            tc.tile_pool(name="accbig", bufs=1) as p_acc,
            tc.tile_pool(name="msg", bufs=8) as p_msg,
            tc.tile_pool(name="sel", bufs=6) as p_sel,
            tc.tile_pool(name="small", bufs=3) as p_small,
            tc.tile_pool(name="dpsum", bufs=2, space="PSUM") as p_dpsum,
            tc.tile_pool(name="epsum", bufs=6, space="PSUM") as p_epsum,
        ):
            W1_t = p_const.tile([D, D], f32)
            W2_t = p_const.tile([D, D], f32)
            b1_t = p_const.tile([128, 1], f32)
            idx_t = p_const.tile([128, t_total * 8], mybir.dt.int16)
            nc.sync.dma_start(out=W1_t[:], in_=W1_d[:])
            nc.sync.dma_start(out=W2_t[:], in_=W2_d[:])
            nc.sync.dma_start(out=b1_t[:], in_=b1_d[:])
            nc.sync.dma_start(out=idx_t[:], in_=idx_d[:])
            if b2_nonzero:
                b2_t = p_const.tile([128, 128], f32)
                nc.sync.dma_start(out=b2_t[:], in_=b2_d[:])

            # ---------- dense 1: h1 = x @ W1 ----------
            with tc.tile_pool(name="xp", bufs=1) as p_x:
                xT_t = p_x.tile([D, SHARD], f32)
                nc.sync.dma_start(out=xT_t[:], in_=xT_d[:])
                for j in range(NBLK):
                    ps = p_dpsum.tile([128, D], f32, space="PSUM", tag="dps")
                    nc.tensor.matmul(out=ps[:], lhsT=xT_t[:, j * 128:(j + 1) * 128],
                                     rhs=W1_t[:], start=True, stop=True)
                    hb = p_small.tile([128, D], f32, tag="hsb")
                    nc.scalar.activation(out=hb[:], in_=ps[:],
                                         func=mybir.ActivationFunctionType.Copy)
                    nc.sync.dma_start(out=h1_bounce[j * 128:(j + 1) * 128, :], in_=hb[:])

            nc.gpsimd.collective_compute(
                "AllGather", mybir.AluOpType.bypass,
                replica_groups=[list(range(NCORES))],
                ins=[h1_bounce[:]], outs=[h1_full[:]],
            )

            acc1 = p_acc.tile([128, SHARD], f32, tag="acc")

            # ---------- edge phase ----------
            def edge_phase(h_bounce, h_full, acc, transposed):
                # iterate gather calls; matmul-accumulate per (w,b) group
                pending = {}
                for ci, (w, t0, nt) in enumerate(calls):
                    src_ap = (h_bounce[:] if w == 0
                              else h_full[(w - 1) * WIN:w * WIN, :])
                    msg_t = p_msg.tile([128, CHUNK_T, D], f32, tag="msg")
                    nc.gpsimd.dma_gather(
                        out_ap=msg_t[:, :nt, :],
                        in_ap=src_ap,
                        idxs_ap=idx_t[:, t0 * 8:(t0 + nt) * 8],
                        num_idxs=nt * 128, num_idxs_reg=nt * 128,
                        elem_size=D, queue_num=ci % 4)
                    S_t = p_sel.tile([128, CHUNK_T * 128], f32, tag="sel")
                    nc.sync.dma_start(out=S_t[:, :nt * 128],
                                      in_=S_d[:, t0 * 128:(t0 + nt) * 128])
                    # run matmuls for all tiles in this chunk
                    for t in range(t0, t0 + nt):
                        # which group is tile t in?
                        g = np.searchsorted(tile_base, t, side="right") - 1
                        gs, ge = tile_base[g], tile_base[g + 1]
                        first, last = (t == gs), (t == ge - 1)
                        b = g % NBLK
                        if first:
                            ps = p_epsum.tile([128, D], f32, space="PSUM", tag="eps")
                            pending[g] = ps
                        ps = pending[g]
                        mt = msg_t[:, t - t0, :]
                        st = S_t[:, (t - t0) * 128:(t - t0 + 1) * 128]
                        if transposed:
                            nc.tensor.matmul(out=ps[:], lhsT=mt, rhs=st,
                                             start=first, stop=last)
                        else:
                            nc.tensor.matmul(out=ps[:], lhsT=st, rhs=mt,
                                             start=first, stop=last)
                        if last:
                            dstsl = acc[:, b * 128:(b + 1) * 128]
                            if g < NBLK:  # window 0: initialize
                                nc.vector.tensor_copy(out=dstsl, in_=ps[:])
                            else:
                                nc.vector.tensor_add(out=dstsl, in0=dstsl, in1=ps[:])
                            del pending[g]

            edge_phase(h1_bounce, h1_full, acc1, transposed=True)

            # epilogue 1 (transposed world): out1T = relu(acc1 + b1)
            for b in range(NBLK):
                sl = acc1[:, b * 128:(b + 1) * 128]
                nc.scalar.activation(out=sl, in_=sl,
                                     func=mybir.ActivationFunctionType.Relu,
                                     bias=b1_t[:, :1])

            # ---------- dense 2: h2 = relu1 @ W2 ----------
            for j in range(NBLK):
                ps = p_dpsum.tile([128, D], f32, space="PSUM", tag="dps")
                nc.tensor.matmul(out=ps[:], lhsT=acc1[:, j * 128:(j + 1) * 128],
                                 rhs=W2_t[:], start=True, stop=True)
                hb = p_small.tile([128, D], f32, tag="hsb")
                nc.scalar.activation(out=hb[:], in_=ps[:],
                                     func=mybir.ActivationFunctionType.Copy)
                nc.sync.dma_start(out=h2_bounce[j * 128:(j + 1) * 128, :], in_=hb[:])

            nc.gpsimd.collective_compute(
                "AllGather", mybir.AluOpType.bypass,
                replica_groups=[list(range(NCORES))],
                ins=[h2_bounce[:]], outs=[h2_full[:]],
            )

            acc2 = p_acc.tile([128, SHARD], f32, tag="acc")
            edge_phase(h2_bounce, h2_full, acc2, transposed=False)

            # epilogue 2 (normal world): out = relu(acc2 [+ b2])
            for b in range(NBLK):
                sl = acc2[:, b * 128:(b + 1) * 128]
                if b2_nonzero:
                    nc.vector.tensor_add(out=sl, in0=sl, in1=b2_t[:])
                ob = p_small.tile([128, D], f32, tag="osb")
                nc.scalar.activation(out=ob[:], in_=sl,
                                     func=mybir.ActivationFunctionType.Relu)
                nc.sync.dma_start(out=out_d[b * 128:(b + 1) * 128, :], in_=ob[:])

    nc.compile()
    return nc


def prepare(x, edge_index, W1, b1, W2, b2):
    """Host prep + (cached) program build. Returns (nc, in_maps, pos_of_node)."""
    in_maps, sched_sig, tile_base, t_total, b2_nonzero, pos_of_node = _host_prep(
        x, edge_index, W1, b1, W2, b2)
    calls = sched_sig[1]
    key = sched_sig
    if key not in _CACHE:
        _CACHE[key] = _build_program(tile_base, t_total, list(calls), b2_nonzero)
    return _CACHE[key], in_maps, pos_of_node


def kernel(x, edge_index, W1, b1, W2, b2):
    nc, in_maps, pos_of_node = prepare(x, edge_index, W1, b1, W2, b2)
    res = run_bass_kernel_spmd(nc, in_maps, list(range(NCORES)))
    full = np.concatenate([res.results[k]["out"] for k in range(NCORES)], axis=0)
    n = np.asarray(x).shape[0]
    return full[pos_of_node[:n]]



# revision 3
# speedup vs baseline: 2.7245x; 2.7245x over previous
"""Two-layer GCN (PyG GCNConv x2 + ReLU) on 8 Trainium2 NeuronCores.

Strategy (dst-sharded SPMD, aggregation-first):
  - GCN layer = relu((A_hat @ x) @ W + b): the dense matmul commutes with the
    aggregation, so each layer gathers rows of the (bf16) node table, scatter-
    adds them via on-device-built selection-matrix matmuls into per-block
    PSUM, then applies the 128x128 dense + ReLU per 128-node block.
  - Layer 1's table is x itself (replicated input) -> no dense pre-pass and no
    collective before layer 1. Self-loop contributions bypass the gather
    entirely: they are a host-prescaled table xts = a^2 * x applied as a
    second accumulating matmul into the dense PSUM.
  - Layer 2's table is out1, exchanged via 4 chunked AllGathers (25 blocks
    each) pipelined against edge-phase compute: chunk c is gathered by layer 2
    as soon as AG_c lands while later chunks are still being produced.
  - Selection matrices S[e, d] = norm_e * (d == dst_e) are built on the DVE
    per tile from compact per-edge metadata (one fused is_equal*mult
    tensor_scalar against a constant bf16 iota tile, f32 scalars to keep the
    DVE 2x 16-bit mode) -- no S-matrix DMA from HBM.
  - Everything bf16 except PSUM accumulation (f32), the layer-2 inter-chunk
    accumulator (f32), and the epilogue math; output written bf16 and
    upcast to f32 on host.
  - Nodes are padded to 102400 and dealt round-robin over the 800
    (core, block) pairs by descending in-degree so per-block edge counts are
    balanced; all device-side structures live in position space.
"""

import numpy as np

import concourse.bass as bass
import concourse.bacc as bacc
import concourse.mybir as mybir
import concourse.tile as tile
from concourse.bass_utils import run_bass_kernel_spmd

N = 100000
E = 640000
D = 128
NCORES = 8
NPAD = 102400
SHARD = NPAD // NCORES        # 12800
NBLK = SHARD // 128           # 100 dst blocks per core
WIN = 25600                   # gather window rows (int16-safe); NPAD/WIN = 4
NWIN = NPAD // WIN            # 4
NCHUNK = 4                    # AllGather chunks for the layer-2 table
CBLK = NBLK // NCHUNK         # 25 blocks per chunk
SUB = 5                       # blocks per layer-1 PSUM sub-chunk (banks)
NSUB = CBLK // SUB            # 5 sub-chunks per chunk
CHUNK_T = 8                   # tiles (128 edges) per dma_gather call

BF = mybir.dt.bfloat16
NPBF = mybir.dt.np(BF)

_CACHE = {}
PADNEG = False   # pad gather slots with -1 (DMA skips them) instead of 0 (crashes HW; keep False)
QUEUES = 4       # SWDGE queues for gather round-robin
SCRATCH = None   # dynamic_dma_scratch_size override


def _group_schedule(gid, rel, nrm, dloc, ngroups, counts_max):
    """Common-tile-schedule packing for one layer on one core.

    Returns flat (per 128-slot tile) gidx/norm/dst arrays in tile order.
    """
    T = (counts_max + 127) // 128
    T = np.maximum(T, 1)
    tile_base = np.zeros(ngroups + 1, np.int64)
    tile_base[1:] = np.cumsum(T)
    t_total = int(tile_base[-1])

    order = np.lexsort((rel, gid))
    gid_s = gid[order]
    counts = np.bincount(gid_s, minlength=ngroups)
    grp_off = np.zeros(ngroups + 1, np.int64)
    grp_off[1:] = np.cumsum(counts)
    rank = np.arange(gid_s.shape[0], dtype=np.int64) - grp_off[gid_s]
    pos = tile_base[gid_s] * 128 + rank

    gidx = np.full(t_total * 128, -1 if PADNEG else 0, np.int16)
    norm = np.zeros(t_total * 128, np.float32)
    dst = np.zeros(t_total * 128, np.int64)
    gidx[pos] = rel[order].astype(np.int16)
    norm[pos] = nrm[order]
    dst[pos] = dloc[order]
    return T, tile_base, t_total, gidx, norm, dst


def _wrap_idx(gidx, calls, t_total):
    idxw = np.zeros((128, t_total * 8), np.int16)
    for (w, t0, nt) in calls:
        blk = gidx[t0 * 128:(t0 + nt) * 128].reshape(nt * 8, 16).T
        idxw[:, t0 * 8:(t0 + nt) * 8] = np.tile(blk, (8, 1))
    return idxw


def _calls_from_runs(tile_base, runs):
    """runs: list of (window, gid_start, gid_end). Chop each run's tile range
    into dma_gather calls of <= CHUNK_T tiles."""
    calls = []
    for (w, g0, g1) in runs:
        t = int(tile_base[g0])
        t_end = int(tile_base[g1])
        while t < t_end:
            nt = min(CHUNK_T, t_end - t)
            calls.append((w, t, nt))
            t += nt
    return calls


def _host_prep(x, edge_index, W1, b1, W2, b2):
    x = np.asarray(x, dtype=np.float32)
    ei = np.asarray(edge_index)
    W1 = np.asarray(W1, dtype=np.float32)
    W2 = np.asarray(W2, dtype=np.float32)
    b1 = np.asarray(b1, dtype=np.float32)
    b2 = np.asarray(b2, dtype=np.float32)
    n = x.shape[0]

    src = np.concatenate([ei[0].astype(np.int64), np.arange(n, dtype=np.int64)])
    dst = np.concatenate([ei[1].astype(np.int64), np.arange(n, dtype=np.int64)])
    deg = np.bincount(dst, minlength=NPAD).astype(np.float32)
    a = np.zeros(NPAD, np.float32)
    nz = deg > 0
    a[nz] = 1.0 / np.sqrt(deg[nz])

    # degree-balanced node->position permutation (nodes dealt round-robin over
    # the 800 (core, block) pairs by descending degree)
    order_by_deg = np.argsort(-deg, kind="stable")
    i = np.arange(NPAD, dtype=np.int64)
    cb = i % (NCORES * NBLK)
    position_of_rank = (cb % NCORES) * SHARD + (cb // NCORES) * 128 + i // (NCORES * NBLK)
    pos_of_node = np.empty(NPAD, np.int64)
    pos_of_node[order_by_deg] = position_of_rank
    node_at_pos = np.empty(NPAD, np.int64)
    node_at_pos[pos_of_node] = i

    ps = pos_of_node[src]
    pd = pos_of_node[dst]
    core = pd // SHARD
    norm_all = a[src] * a[dst]
    is_self = np.zeros(src.shape[0], bool)
    is_self[E:] = True          # the appended self-loops

    x_pad = np.zeros((NPAD, D), np.float32)
    x_pad[:n] = x
    x_perm = x_pad[node_at_pos]
    x_perm_bf = np.ascontiguousarray(x_perm.astype(NPBF))
    a_pos = a[node_at_pos]
    # per-position self-loop contribution table: x[d] * a_d^2, feature-major
    xts_full = (x_perm * (a_pos ** 2)[:, None]).astype(NPBF)

    NG1 = NCHUNK * NSUB * NWIN * SUB     # 400
    NG2 = NCHUNK * NBLK                  # 400

    per_core = []
    cmax1 = np.zeros(NG1, np.int64)
    cmax2 = np.zeros(NG2, np.int64)
    for k in range(NCORES):
        m = core == k
        m1 = m & ~is_self               # layer 1 skips self-loops (handled densely)
        s1k = ps[m1]
        d1k = pd[m1]
        nrm1 = norm_all[m1]
        blk1 = (d1k % SHARD) // 128
        dloc1 = d1k % 128
        c1 = blk1 // CBLK
        ss1 = (blk1 % CBLK) // SUB
        b5 = blk1 % SUB
        w1 = s1k // WIN
        gid1 = ((c1 * NSUB + ss1) * NWIN + w1) * SUB + b5
        rel1 = s1k - w1 * WIN

        s_k = ps[m]
        d_k = pd[m]
        nrm = norm_all[m]
        blk = (d_k % SHARD) // 128
        dloc = d_k % 128
        sb = (s_k % SHARD) // 128        # source block 0..99
        c2 = sb // CBLK
        rel2 = (s_k // SHARD) * (CBLK * 128) + (sb % CBLK) * 128 + (s_k % 128)
        gid2 = c2 * NBLK + blk

        cmax1 = np.maximum(cmax1, np.bincount(gid1, minlength=NG1))
        cmax2 = np.maximum(cmax2, np.bincount(gid2, minlength=NG2))
        per_core.append((gid1, rel1, nrm1, dloc1, gid2, rel2, nrm, dloc))

    T1 = (cmax1 + 127) // 128
    T1 = np.maximum(T1, 1)
    tb1 = np.zeros(NG1 + 1, np.int64)
    tb1[1:] = np.cumsum(T1)
    t1 = int(tb1[-1])
    T2 = (cmax2 + 127) // 128
    T2 = np.maximum(T2, 1)
    tb2 = np.zeros(NG2 + 1, np.int64)
    tb2[1:] = np.cumsum(T2)
    t2 = int(tb2[-1])

    # gather-call schedule (identical across cores)
    runs1 = []
    for c in range(NCHUNK):
        for s in range(NSUB):
            for w in range(NWIN):
                g0 = ((c * NSUB + s) * NWIN + w) * SUB
                runs1.append((w, g0, g0 + SUB))
    calls1 = _calls_from_runs(tb1, runs1)
    runs2 = [(c, c * NBLK, (c + 1) * NBLK) for c in range(NCHUNK)]
    calls2 = _calls_from_runs(tb2, runs2)

    in_maps = []
    b1nz = bool(np.any(b1 != 0.0))
    b2nz = bool(np.any(b2 != 0.0))
    for k in range(NCORES):
        gid1, rel1, nrm1, dloc1, gid2, rel2, nrm, dloc = per_core[k]
        _, _, _, gidx1, norm1, dst1 = _group_schedule(
            gid1, rel1, nrm1, dloc1, NG1, cmax1)
        _, _, _, gidx2, norm2, dst2 = _group_schedule(
            gid2, rel2, nrm, dloc, NG2, cmax2)
        im = {
            "x": x_perm_bf,
            "xts": np.ascontiguousarray(
                xts_full[k * SHARD:(k + 1) * SHARD].T),
            "idx1": _wrap_idx(gidx1, calls1, t1),
            "md1": np.ascontiguousarray(dst1.reshape(t1, 128).T.astype(np.float32)),
            "mn1": np.ascontiguousarray(norm1.reshape(t1, 128).T.astype(np.float32)),
            "idx2": _wrap_idx(gidx2, calls2, t2),
            "md2": np.ascontiguousarray(dst2.reshape(t2, 128).T.astype(np.float32)),
            "mn2": np.ascontiguousarray(norm2.reshape(t2, 128).T.astype(np.float32)),
            "W1": W1.astype(NPBF),
            "W2": W2.astype(NPBF),
            "b1bc": np.broadcast_to(b1, (128, 128)).astype(np.float32).copy(),
            "b2bc": np.broadcast_to(b2, (128, 128)).astype(np.float32).copy(),
        }
        in_maps.append(im)

    sched = {
        "T1": tuple(int(v) for v in T1),
        "T2": tuple(int(v) for v in T2),
        "calls1": tuple(calls1),
        "calls2": tuple(calls2),
        "t1": t1,
        "t2": t2,
        "b1nz": b1nz,
        "b2nz": b2nz,
    }
    return in_maps, sched, pos_of_node


def _build_program(sched, variant="full"):
    T1 = np.array(sched["T1"], np.int64)
    T2 = np.array(sched["T2"], np.int64)
    tb1 = np.zeros(T1.shape[0] + 1, np.int64)
    tb1[1:] = np.cumsum(T1)
    tb2 = np.zeros(T2.shape[0] + 1, np.int64)
    tb2[1:] = np.cumsum(T2)
    calls1 = list(sched["calls1"])
    calls2 = list(sched["calls2"])
    t1, t2 = sched["t1"], sched["t2"]
    b1nz, b2nz = sched["b1nz"], sched["b2nz"]

    # per-tile annotations, layer 1: block id + first/last flags
    # gid1 -> (c, s, w, b5); block = (c*NSUB + s)*SUB + b5
    blk_of_t1 = np.zeros(t1, np.int64)
    first_t1 = np.zeros(t1, bool)
    last_t1 = np.zeros(t1, bool)
    for g in range(T1.shape[0]):
        b5 = g % SUB
        w = (g // SUB) % NWIN
        sc = g // (SUB * NWIN)           # c*NSUB + s
        b = sc * SUB + b5
        blk_of_t1[tb1[g]:tb1[g + 1]] = b
        if w == 0:
            first_t1[tb1[g]] = True
        if w == NWIN - 1:
            last_t1[tb1[g + 1] - 1] = True
    # layer 2: gid2 = c*NBLK + b
    blk_of_t2 = np.zeros(t2, np.int64)
    c_of_t2 = np.zeros(t2, np.int64)
    first_t2 = np.zeros(t2, bool)
    last_t2 = np.zeros(t2, bool)
    for g in range(T2.shape[0]):
        c = g // NBLK
        b = g % NBLK
        blk_of_t2[tb2[g]:tb2[g + 1]] = b
        c_of_t2[tb2[g]:tb2[g + 1]] = c
        first_t2[tb2[g]] = True
        last_t2[tb2[g + 1] - 1] = True

    kw = {}
    if SCRATCH is not None:
        kw["dynamic_dma_scratch_size"] = SCRATCH
    nc = bacc.Bacc("TRN2", target_bir_lowering=False, debug=False,
                   num_devices=NCORES, num_swdge_queues=QUEUES, **kw)
    f32 = mybir.dt.float32
    i16 = mybir.dt.int16

    x_d = nc.dram_tensor("x", [NPAD, D], BF, kind="ExternalInput")
    xts_d = nc.dram_tensor("xts", [D, SHARD], BF, kind="ExternalInput")
    idx1_d = nc.dram_tensor("idx1", [128, t1 * 8], i16, kind="ExternalInput")
    md1_d = nc.dram_tensor("md1", [128, t1], f32, kind="ExternalInput")
    mn1_d = nc.dram_tensor("mn1", [128, t1], f32, kind="ExternalInput")
    idx2_d = nc.dram_tensor("idx2", [128, t2 * 8], i16, kind="ExternalInput")
    md2_d = nc.dram_tensor("md2", [128, t2], f32, kind="ExternalInput")
    mn2_d = nc.dram_tensor("mn2", [128, t2], f32, kind="ExternalInput")
    W1_d = nc.dram_tensor("W1", [D, D], BF, kind="ExternalInput")
    W2_d = nc.dram_tensor("W2", [D, D], BF, kind="ExternalInput")
    b1_d = nc.dram_tensor("b1bc", [128, 128], f32, kind="ExternalInput")
    b2_d = nc.dram_tensor("b2bc", [128, 128], f32, kind="ExternalInput")
    out_d = nc.dram_tensor("out", [SHARD, D], BF, kind="ExternalOutput")

    ob = [nc.dram_tensor(f"ob{c}", [CBLK * 128, D], BF) for c in range(NCHUNK)]
    hf = [nc.dram_tensor(f"hf{c}", [CBLK * 128 * NCORES, D], BF,
                         addr_space="Shared") for c in range(NCHUNK)]

    with tile.TileContext(nc) as tc:
        with (
            tc.tile_pool(name="const", bufs=1) as p_const,
            tc.tile_pool(name="acc", bufs=1) as p_acc,
            tc.tile_pool(name="msg", bufs=8) as p_msg,
            tc.tile_pool(name="sel", bufs=8) as p_sel,
            tc.tile_pool(name="agg", bufs=4) as p_agg,
            tc.tile_pool(name="o1", bufs=4) as p_o1,
            tc.tile_pool(name="tmp", bufs=2) as p_tmp,
            tc.tile_pool(name="eps", bufs=SUB, space="PSUM") as p_eps,
            tc.tile_pool(name="dps", bufs=2, space="PSUM") as p_dps,
        ):
            W1_t = p_const.tile([D, D], BF)
            W2_t = p_const.tile([D, D], BF)
            idx1_t = p_const.tile([128, t1 * 8], i16)
            md1_t = p_const.tile([128, t1], f32)
            mn1_t = p_const.tile([128, t1], f32)
            idx2_t = p_const.tile([128, t2 * 8], i16)
            md2_t = p_const.tile([128, t2], f32)
            mn2_t = p_const.tile([128, t2], f32)
            nc.sync.dma_start(out=W1_t[:], in_=W1_d[:])
            nc.sync.dma_start(out=W2_t[:], in_=W2_d[:])
            nc.sync.dma_start(out=idx1_t[:], in_=idx1_d[:])
            nc.sync.dma_start(out=md1_t[:], in_=md1_d[:])
            nc.sync.dma_start(out=mn1_t[:], in_=mn1_d[:])
            nc.sync.dma_start(out=idx2_t[:], in_=idx2_d[:])
            nc.sync.dma_start(out=md2_t[:], in_=md2_d[:])
            nc.sync.dma_start(out=mn2_t[:], in_=mn2_d[:])
            if b1nz or b2nz:
                b1_t = p_const.tile([128, 128], f32)
                b2_t = p_const.tile([128, 128], f32)
                nc.sync.dma_start(out=b1_t[:], in_=b1_d[:])
                nc.sync.dma_start(out=b2_t[:], in_=b2_d[:])

            xts_t = p_const.tile([D, SHARD], BF)
            nc.sync.dma_start(out=xts_t[:], in_=xts_d[:])
            iota_i = p_const.tile([128, 128], mybir.dt.int32)
            iota_bf = p_const.tile([128, 128], BF)
            nc.gpsimd.iota(out=iota_i[:], pattern=[[1, 128]], base=0,
                           channel_multiplier=0)
            nc.vector.tensor_copy(out=iota_bf[:], in_=iota_i[:])

            acc2 = p_acc.tile([128, SHARD], f32)

            if PADNEG:
                for _z in range(8):
                    zt = p_msg.tile([128, CHUNK_T, D], BF, tag="msg", name="zmsg")
                    nc.vector.memset(zt[:], 0.0)

            def build_S(md_t, mn_t, t):
                S_t = p_sel.tile([128, 128], BF, tag="sel")
                nc.vector.tensor_scalar(
                    out=S_t[:], in0=iota_bf[:],
                    scalar1=md_t[:, t:t + 1], scalar2=mn_t[:, t:t + 1],
                    op0=mybir.AluOpType.is_equal, op1=mybir.AluOpType.mult)
                return S_t

            # ---------------- layer 1 ----------------
            pending = {}
            chunk_done_emitted = [False] * NCHUNK
            for ci, (w, t0, nt) in enumerate(calls1):
                msg_t = p_msg.tile([128, CHUNK_T, D], BF, tag="msg")
                nc.gpsimd.dma_gather(
                    out_ap=msg_t[:, :nt, :],
                    in_ap=x_d[w * WIN:(w + 1) * WIN, :],
                    idxs_ap=idx1_t[:, t0 * 8:(t0 + nt) * 8],
                    num_idxs=nt * 128, num_idxs_reg=nt * 128,
                    elem_size=D, queue_num=ci % QUEUES)
                for t in range(t0, t0 + nt):
                    b = int(blk_of_t1[t])
                    S_t = build_S(md1_t, mn1_t, t)
                    if first_t1[t]:
                        pending[b] = p_eps.tile([128, D], f32, space="PSUM",
                                                tag="eps", name="eps")
                    ps = pending[b]
                    nc.tensor.matmul(out=ps[:], lhsT=msg_t[:, t - t0, :],
                                     rhs=S_t[:], start=bool(first_t1[t]),
                                     stop=bool(last_t1[t]))
                    if last_t1[t]:
                        del pending[b]
                        aggbf = p_agg.tile([128, D], BF, tag="agg")
                        nc.scalar.activation(
                            out=aggbf[:], in_=ps[:],
                            func=mybir.ActivationFunctionType.Copy)
                        ps2 = p_dps.tile([128, D], f32, space="PSUM", tag="dps")
                        nc.tensor.matmul(out=ps2[:], lhsT=aggbf[:], rhs=W1_t[:],
                                         start=True, stop=False)
                        nc.tensor.matmul(
                            out=ps2[:], lhsT=xts_t[:, b * 128:(b + 1) * 128],
                            rhs=W1_t[:], start=False, stop=True)
                        o1 = p_o1.tile([128, D], BF, tag="o1")
                        if b1nz:
                            tmp = p_tmp.tile([128, D], f32, tag="tmp")
                            nc.vector.tensor_add(out=tmp[:], in0=ps2[:],
                                                 in1=b1_t[:])
                            nc.scalar.activation(
                                out=o1[:], in_=tmp[:],
                                func=mybir.ActivationFunctionType.Relu)
                        else:
                            nc.scalar.activation(
                                out=o1[:], in_=ps2[:],
                                func=mybir.ActivationFunctionType.Relu)
                        c = b // CBLK
                        nc.sync.dma_start(
                            out=ob[c][(b % CBLK) * 128:(b % CBLK + 1) * 128, :],
                            in_=o1[:])
                        # AllGather chunk c once its 25 blocks are all evicted
                        if (b % CBLK) == CBLK - 1 and variant == "full":
                            assert not chunk_done_emitted[c]
                            chunk_done_emitted[c] = True
                            nc.gpsimd.collective_compute(
                                "AllGather", mybir.AluOpType.bypass,
                                replica_groups=[list(range(NCORES))],
                                ins=[ob[c][:]], outs=[hf[c][:]],
                            )

            # ---------------- layer 2 ----------------
            pending2 = {}
            for ci, (c, t0, nt) in enumerate(calls2):
                msg_t = p_msg.tile([128, CHUNK_T, D], BF, tag="msg")
                src_ap = hf[c][:] if variant == "full" else x_d[0:CBLK * 128 * NCORES, :]
                nc.gpsimd.dma_gather(
                    out_ap=msg_t[:, :nt, :],
                    in_ap=src_ap,
                    idxs_ap=idx2_t[:, t0 * 8:(t0 + nt) * 8],
                    num_idxs=nt * 128, num_idxs_reg=nt * 128,
                    elem_size=D, queue_num=ci % QUEUES)
                for t in range(t0, t0 + nt):
                    b = int(blk_of_t2[t])
                    S_t = build_S(md2_t, mn2_t, t)
                    if first_t2[t]:
                        pending2[b] = p_eps.tile([128, D], f32, space="PSUM",
                                                 tag="eps", name="eps")
                    ps = pending2[b]
                    nc.tensor.matmul(out=ps[:], lhsT=msg_t[:, t - t0, :],
                                     rhs=S_t[:], start=bool(first_t2[t]),
                                     stop=bool(last_t2[t]))
                    if last_t2[t]:
                        del pending2[b]
                        accblk = acc2[:, b * 128:(b + 1) * 128]
                        if c == 0:
                            nc.scalar.activation(
                                out=accblk, in_=ps[:],
                                func=mybir.ActivationFunctionType.Copy)
                        elif c < NCHUNK - 1:
                            nc.vector.tensor_add(out=accblk, in0=accblk,
                                                 in1=ps[:])
                        else:
                            agg2 = p_agg.tile([128, D], BF, tag="agg")
                            nc.vector.tensor_add(out=agg2[:], in0=accblk,
                                                 in1=ps[:])
                            ps2 = p_dps.tile([128, D], f32, space="PSUM",
                                             tag="dps")
                            nc.tensor.matmul(out=ps2[:], lhsT=agg2[:],
                                             rhs=W2_t[:], start=True, stop=True)
                            ot = p_o1.tile([128, D], BF, tag="o1")
                            if b2nz:
                                tmp = p_tmp.tile([128, D], f32, tag="tmp")
                                nc.vector.tensor_add(out=tmp[:], in0=ps2[:],
                                                     in1=b2_t[:])
                                nc.scalar.activation(
                                    out=ot[:], in_=tmp[:],
                                    func=mybir.ActivationFunctionType.Relu)
                            else:
                                nc.scalar.activation(
                                    out=ot[:], in_=ps2[:],
                                    func=mybir.ActivationFunctionType.Relu)
                            nc.sync.dma_start(
                                out=out_d[b * 128:(b + 1) * 128, :], in_=ot[:])

    nc.compile()
    return nc


def prepare(x, edge_index, W1, b1, W2, b2, variant="full"):
    in_maps, sched, pos_of_node = _host_prep(x, edge_index, W1, b1, W2, b2)
    key = (sched["T1"], sched["T2"], sched["calls1"], sched["calls2"],
           sched["b1nz"], sched["b2nz"], variant, PADNEG, QUEUES, SCRATCH)
    if key not in _CACHE:
        _CACHE[key] = _build_program(sched, variant)
    return _CACHE[key], in_maps, pos_of_node


def kernel(x, edge_index, W1, b1, W2, b2):
    nc, in_maps, pos_of_node = prepare(x, edge_index, W1, b1, W2, b2)
    res = run_bass_kernel_spmd(nc, in_maps, list(range(NCORES)))
    full = np.concatenate([res.results[k]["out"] for k in range(NCORES)], axis=0)
    n = np.asarray(x).shape[0]
    return np.ascontiguousarray(full[pos_of_node[:n]].astype(np.float32))
